# revision 4
# baseline (speedup 1.0000x reference)
"""ForgetMult (h_t = f_t*h_{t-1} + (1-f_t)*z_t) on 8 TRN2 NeuronCores.

Full inputs f, z: [T=1024, B=32, H=1024] f32. Output h: [T, B, H] f32.

Sharding: batch dim across the 8 cores (4 batches/core), no communication.
Per core: independent linear recurrence along T for N = 4096 columns.

v4 strategy (engine balance via the shift-subtract identity):
    d_t = z_{t-1} - z_t                 (elementwise)
    s_t = (d_t + s_{t-1}) * f_t         (the canned DVE scan: op0=add,
                                         op1=mult -- no precomputed b!)
    h_t = s_t + z_t                     (elementwise)
  This splits the 3 DVE-cycles/elem of the naive form (STT 1cyc + scan
  2cyc) across three engines:
    DVE:  d (tensor_tensor sub, fp16 2x mode, 0.5 cyc/elem) + scan
          (2 cyc/elem, the irreducible serial part)
    PE :  h = I.T@s + I.T@z accumulated into PSUM (identity matmuls)
    ACT:  PSUM -> SBUF fp16 downcast copy
  HBM: host uploads f, z transposed to [N, T] fp16 and reads h back
  [N, T] fp16 (24 MiB/core, ~72 us at ~349 GB/s/core measured).

  Host t=0 fixup (f'_0 = 0, z'_0 = (1-f_0)*z_0) makes every column
  start self-resetting: s resets to 0 through f'_0 = 0, h_0 = z'_0 is
  exact, and d_1 = z'_0 - z_1 carries h_0 into s_1 correctly. So scans
  chain across n-blocks in one instruction and the shifted d-read needs
  no boundary handling (d at a column start is multiplied by 0).

Precision: fp16 I/O quantization, fp32 scan state / PSUM accumulate
-> rel err ~4e-4, well under the 2e-2 gate.
"""

from contextlib import ExitStack

import numpy as np

T, B, H = 1024, 32, 1024
NCORES = 8
BPC = B // NCORES  # 4 batches per core
N = BPC * H  # 4096 recurrence columns per core
P = 128

J = 4  # n-blocks per group
W = J * T  # free elems per group tile
MM = 512  # PE moving-dim / PSUM chunk width


def build_forget_mult(tc, h_d, f_d, z_d, i_d, ctx):
    """Emit the per-core Tile program. f_d/z_d/h_d are DRAM APs [N, T] fp16."""
    from concourse import mybir

    nc = tc.nc
    f16 = mybir.dt.float16
    fp32 = mybir.dt.float32
    ad = mybir.AluOpType.add
    su = mybir.AluOpType.subtract
    mu = mybir.AluOpType.mult

    ngroups = N // (P * J)  # 8

    const_pool = ctx.enter_context(tc.tile_pool(name="const", bufs=1))
    ident = const_pool.tile([P, P], f16)
    nc.sync.dma_start(ident[:], i_d[:])

    f_pool = ctx.enter_context(tc.tile_pool(name="fpanel", bufs=3))
    z_pool = ctx.enter_context(tc.tile_pool(name="zpanel", bufs=3))
    d_pool = ctx.enter_context(tc.tile_pool(name="dpanel", bufs=2))
    s_pool = ctx.enter_context(tc.tile_pool(name="spanel", bufs=2))
    h_pool = ctx.enter_context(tc.tile_pool(name="hpanel", bufs=2))
    psum_pool = ctx.enter_context(tc.tile_pool(name="hpsum", bufs=4, space="PSUM"))

    def group_dram(d, g):
        # rows [P*J*g : P*J*(g+1)] of [N, T] viewed as [p, j, t]
        return d[P * J * g : P * J * (g + 1), :].rearrange(
            "(j p) t -> p j t", p=P
        )

    for g in range(ngroups):
        fp = f_pool.tile([P, J, T], f16, tag="fpanel")
        nc.sync.dma_start(fp[:], group_dram(f_d, g))
        zp = z_pool.tile([P, J, T], f16, tag="zpanel")
        nc.sync.dma_start(zp[:], group_dram(z_d, g))
        zf = zp[:].rearrange("p j t -> p (j t)")  # [P, W]
        ff = fp[:].rearrange("p j t -> p (j t)")

        # d_t = z_{t-1} - z_t  (t=0 element memset; every column-start d is
        # multiplied by f'=0 in the scan so cross-column leakage is dead)
        dp = d_pool.tile([P, W], f16, tag="dpanel")
        nc.vector.memset(dp[:, 0:1], 0.0)
        nc.vector.tensor_tensor(dp[:, 1:W], zf[:, 0 : W - 1], zf[:, 1:W], op=su)

        # s_t = (d_t + s_{t-1}) * f_t ; fp32 internal state, chained over
        # the whole group (column starts self-reset via f'=0)
        sp = s_pool.tile([P, W], f16, tag="spanel")
        nc.vector.tensor_tensor_scan(sp[:], dp[:], ff, 0.0, op0=ad, op1=mu)

        # h = s + z on PE (identity matmuls accumulating in PSUM), then
        # ACT downcasts PSUM fp32 -> SBUF fp16
        hp = h_pool.tile([P, J, T], f16, tag="hpanel")
        hf = hp[:].rearrange("p j t -> p (j t)")
        for c in range(W // MM):
            cs = slice(c * MM, (c + 1) * MM)
            hq = psum_pool.tile([P, MM], fp32, tag="hpsum")
            nc.tensor.matmul(hq[:], ident[:], sp[:, cs], start=True, stop=False)
            nc.tensor.matmul(hq[:], ident[:], zf[:, cs], start=False, stop=True)
            nc.scalar.copy(hf[:, cs], hq[:])
        nc.sync.dma_start(group_dram(h_d, g), hp[:])


def build_program():
    import concourse.tile as tile
    from concourse import bacc, mybir

    nc = bacc.Bacc(
        "TRN2",
        target_bir_lowering=False,
        debug=False,
        enable_asserts=False,
        num_devices=NCORES,
    )
    f16 = mybir.dt.float16
    f_d = nc.dram_tensor("f", [N, T], f16, kind="ExternalInput").ap()
    z_d = nc.dram_tensor("z", [N, T], f16, kind="ExternalInput").ap()
    i_d = nc.dram_tensor("ident", [P, P], f16, kind="ExternalInput").ap()
    h_d = nc.dram_tensor("h", [N, T], f16, kind="ExternalOutput").ap()
    with tile.TileContext(nc) as tc:
        with ExitStack() as ctx:
            build_forget_mult(tc, h_d, f_d, z_d, i_d, ctx)
    nc.compile()
    return nc


_compiled = None


def _get_program():
    global _compiled
    if _compiled is None:
        _compiled = build_program()
    return _compiled


def kernel(f, z, _trace=False):
    from concourse.bass_utils import run_bass_kernel_spmd

    f = np.asarray(f, dtype=np.float32)
    z = np.asarray(z, dtype=np.float32)
    assert f.shape == (T, B, H) and z.shape == (T, B, H)

    nc = _get_program()
    ident = np.eye(P, dtype=np.float16)
    in_maps = []
    for c in range(NCORES):
        # [T, BPC, H] -> [T, N] -> transpose -> [N, T], downcast to fp16
        fc = f[:, c * BPC : (c + 1) * BPC, :].reshape(T, N).T
        zc = z[:, c * BPC : (c + 1) * BPC, :].reshape(T, N).T
        fc16 = np.ascontiguousarray(fc, dtype=np.float16)
        zc16 = np.ascontiguousarray(zc, dtype=np.float16)
        # t=0 fixup (see module docstring)
        z0 = (1.0 - fc[:, 0]) * zc[:, 0]  # fp32 math
        fc16[:, 0] = np.float16(0.0)
        zc16[:, 0] = z0.astype(np.float16)
        in_maps.append({"f": fc16, "z": zc16, "ident": ident})

    kres = run_bass_kernel_spmd(nc, in_maps, list(range(NCORES)), trace=_trace)
    out = np.empty((T, B, H), dtype=np.float32)
    for c in range(NCORES):
        hc = kres.results[c]["h"]  # [N, T] fp16
        out[:, c * BPC : (c + 1) * BPC, :] = (
            hc.astype(np.float32).reshape(BPC, H, T).transpose(2, 0, 1)
        )
    if _trace:
        return out, kres
    return out


# revision 5
# speedup vs baseline: 1.2013x; 1.2013x over previous
"""ForgetMult (h_t = f_t*h_{t-1} + (1-f_t)*z_t) on 8 TRN2 NeuronCores.

Full inputs f, z: [T=1024, B=32, H=1024] f32. Output h: [T, B, H] f32.

Sharding: batch dim across the 8 cores (4 batches/core), no communication.
Per core: independent linear recurrence along T for N = 4096 columns.

v5: all-DVE pipeline (measured: PE at low pstate + ACT copies + extra
sync cost MORE than they save; GPSIMD rejects TensorScalarPtr ops).
  - HOST uploads f, z transposed to [N, T] fp16, reads h back [N, T]
    fp16 (24 MiB/core ~= 72 us DMA floor at the ~349 GB/s/core measured)
  - per group: STT bneg = (f-1)*z (1.07 ns/elem) + per-block scans
    (2.2 ns/elem, len-1024 measured fastest)
  - group sizes ramp 1,1,2,4,...,4,2,1,1 to shrink pipeline head/tail
  - t=0 host fixup (f'_0=0, z'_0=(1-f_0)z_0) keeps every column
    self-resetting so scans may chain across blocks when profitable
  - groups 5/6/7 carry scan-configuration probes (fp32 data0 / chained
    2048 / all-fp32) to measure dtype/length effects on the scan rate

Precision: fp16 I/O quantization, fp32 scan state -> rel err ~5e-4.
"""

from contextlib import ExitStack

import numpy as np

T, B, H = 1024, 32, 1024
NCORES = 8
BPC = B // NCORES  # 4 batches per core
N = BPC * H  # 4096 recurrence columns per core
P = 128

JMAX = 4
GROUPS = [1, 1, 2, 4, 4, 4, 4, 4, 4, 2, 1, 1]  # blocks per group, sum 32
assert sum(GROUPS) == N // P


def build_forget_mult(tc, h_d, f_d, z_d, ctx):
    """Emit the per-core Tile program. f_d/z_d/h_d are DRAM APs [N, T] fp16."""
    from concourse import mybir

    nc = tc.nc
    f16 = mybir.dt.float16
    f32 = mybir.dt.float32
    su = mybir.AluOpType.subtract
    mu = mybir.AluOpType.mult

    f_pool = ctx.enter_context(tc.tile_pool(name="fpanel", bufs=4))
    z_pool = ctx.enter_context(tc.tile_pool(name="zpanel", bufs=4))
    b_pool = ctx.enter_context(tc.tile_pool(name="bpanel", bufs=3))
    h_pool = ctx.enter_context(tc.tile_pool(name="hpanel", bufs=3))
    b32_pool = ctx.enter_context(tc.tile_pool(name="b32", bufs=2))
    f32_pool = ctx.enter_context(tc.tile_pool(name="f32", bufs=1))
    h32_pool = ctx.enter_context(tc.tile_pool(name="h32", bufs=1))

    def group_dram(d, r0, gj):
        # rows [P*r0 : P*(r0+gj)] of [N, T] viewed as [p, j, t]
        return d[P * r0 : P * (r0 + gj), :].rearrange("(j p) t -> p j t", p=P)

    r0 = 0
    for g, gj in enumerate(GROUPS):
        fp = f_pool.tile([P, gj, T], f16, tag="fpanel", name=f"fp{g}")
        nc.sync.dma_start(fp[:], group_dram(f_d, r0, gj))
        zp = z_pool.tile([P, gj, T], f16, tag="zpanel", name=f"zp{g}")
        nc.sync.dma_start(zp[:], group_dram(z_d, r0, gj))
        hp = h_pool.tile([P, gj, T], f16, tag="hpanel", name=f"hp{g}")

        if g == 5:
            # PROBE 1: scan with fp32 data0 (bneg in fp32), fp16 data1/out
            b32 = b32_pool.tile([P, gj, T], f32, tag="b32", name=f"b32_{g}")
            nc.vector.scalar_tensor_tensor(
                b32[:], fp[:], 1.0, zp[:], op0=su, op1=mu
            )
            for j in range(gj):
                nc.vector.tensor_tensor_scan(
                    hp[:, j], fp[:, j], b32[:, j], 0.0, op0=mu, op1=su
                )
        elif g == 6:
            # PROBE 2: chained scans, len 2048 (column starts self-reset
            # via the host t=0 fixup)
            bp = b_pool.tile([P, gj, T], f16, tag="bpanel", name=f"bp{g}")
            nc.vector.scalar_tensor_tensor(bp[:], fp[:], 1.0, zp[:], op0=su, op1=mu)
            for j0 in range(0, gj, 2):
                nc.vector.tensor_tensor_scan(
                    hp[:, j0 : j0 + 2].rearrange("p j t -> p (j t)"),
                    fp[:, j0 : j0 + 2].rearrange("p j t -> p (j t)"),
                    bp[:, j0 : j0 + 2].rearrange("p j t -> p (j t)"),
                    0.0,
                    op0=mu,
                    op1=su,
                )
        elif g == 7:
            # PROBE 3: all-fp32 scan (f upcast via ACT, fp32 bneg and out)
            b32 = b32_pool.tile([P, gj, T], f32, tag="b32", name=f"b32_{g}")
            nc.vector.scalar_tensor_tensor(
                b32[:], fp[:], 1.0, zp[:], op0=su, op1=mu
            )
            fw = f32_pool.tile([P, gj, T], f32, tag="f32", name=f"f32_{g}")
            nc.scalar.copy(fw[:], fp[:])
            hw = h32_pool.tile([P, gj, T], f32, tag="h32", name=f"h32_{g}")
            for j in range(gj):
                nc.vector.tensor_tensor_scan(
                    hw[:, j], fw[:, j], b32[:, j], 0.0, op0=mu, op1=su
                )
            nc.scalar.copy(hp[:], hw[:])
        else:
            # steady state: STT + per-block scans
            bp = b_pool.tile([P, gj, T], f16, tag="bpanel", name=f"bp{g}")
            nc.vector.scalar_tensor_tensor(bp[:], fp[:], 1.0, zp[:], op0=su, op1=mu)
            for j in range(gj):
                # state = (f * state) - bneg == f*state + (1-f)*z
                nc.vector.tensor_tensor_scan(
                    hp[:, j], fp[:, j], bp[:, j], 0.0, op0=mu, op1=su
                )
        nc.sync.dma_start(group_dram(h_d, r0, gj), hp[:])
        r0 += gj


def build_program():
    import concourse.tile as tile
    from concourse import bacc, mybir

    nc = bacc.Bacc(
        "TRN2",
        target_bir_lowering=False,
        debug=False,
        enable_asserts=False,
        num_devices=NCORES,
    )
    f16 = mybir.dt.float16
    f_d = nc.dram_tensor("f", [N, T], f16, kind="ExternalInput").ap()
    z_d = nc.dram_tensor("z", [N, T], f16, kind="ExternalInput").ap()
    h_d = nc.dram_tensor("h", [N, T], f16, kind="ExternalOutput").ap()
    with tile.TileContext(nc) as tc:
        with ExitStack() as ctx:
            build_forget_mult(tc, h_d, f_d, z_d, ctx)
    nc.compile()
    return nc


_compiled = None


def _get_program():
    global _compiled
    if _compiled is None:
        _compiled = build_program()
    return _compiled


def kernel(f, z, _trace=False):
    from concourse.bass_utils import run_bass_kernel_spmd

    f = np.asarray(f, dtype=np.float32)
    z = np.asarray(z, dtype=np.float32)
    assert f.shape == (T, B, H) and z.shape == (T, B, H)

    nc = _get_program()
    in_maps = []
    for c in range(NCORES):
        # [T, BPC, H] -> [T, N] -> transpose -> [N, T], downcast to fp16
        fc = f[:, c * BPC : (c + 1) * BPC, :].reshape(T, N).T
        zc = z[:, c * BPC : (c + 1) * BPC, :].reshape(T, N).T
        fc16 = np.ascontiguousarray(fc, dtype=np.float16)
        zc16 = np.ascontiguousarray(zc, dtype=np.float16)
        # t=0 fixup: f'_0 = 0, z'_0 = (1-f_0)*z_0: bneg_0 = -(1-f_0)z_0 and
        # h_0 = 0*carry - bneg_0 is exact for any carried state, so scans
        # may chain across column boundaries.
        z0 = (1.0 - fc[:, 0]) * zc[:, 0]  # fp32 math
        fc16[:, 0] = np.float16(0.0)
        zc16[:, 0] = z0.astype(np.float16)
        in_maps.append({"f": fc16, "z": zc16})

    kres = run_bass_kernel_spmd(nc, in_maps, list(range(NCORES)), trace=_trace)
    out = np.empty((T, B, H), dtype=np.float32)
    for c in range(NCORES):
        hc = kres.results[c]["h"]  # [N, T] fp16
        out[:, c * BPC : (c + 1) * BPC, :] = (
            hc.astype(np.float32).reshape(BPC, H, T).transpose(2, 0, 1)
        )
    if _trace:
        return out, kres
    return out


# revision 8
# speedup vs baseline: 1.2147x; 1.0112x over previous
"""ForgetMult (h_t = f_t*h_{t-1} + (1-f_t)*z_t) on 8 TRN2 NeuronCores.

Full inputs f, z: [T=1024, B=32, H=1024] f32. Output h: [T, B, H] f32.

Sharding: batch dim across the 8 cores (4 batches/core), no communication.
Per core: independent linear recurrence along T for N = 4096 columns.

v5: all-DVE pipeline (measured: PE at low pstate + ACT copies + extra
sync cost MORE than they save; GPSIMD rejects TensorScalarPtr ops).
  - HOST uploads f, z transposed to [N, T] fp16, reads h back [N, T]
    fp16 (24 MiB/core ~= 72 us DMA floor at the ~349 GB/s/core measured)
  - per group: STT bneg = (f-1)*z (1.07 ns/elem) + per-block scans
    (2.2 ns/elem, len-1024 measured fastest)
  - group sizes ramp 1,1,2,4,...,4,2,1,1 to shrink pipeline head/tail
  - t=0 host fixup (f'_0=0, z'_0=(1-f_0)z_0) keeps every column
    self-resetting so scans may chain across blocks when profitable
  - groups 5/6/7 carry scan-configuration probes (fp32 data0 / chained
    2048 / all-fp32) to measure dtype/length effects on the scan rate

Precision: fp16 I/O quantization, fp32 scan state -> rel err ~5e-4.
"""

from contextlib import ExitStack

import numpy as np

T, B, H = 1024, 32, 1024
NCORES = 8
BPC = B // NCORES  # 4 batches per core
N = BPC * H  # 4096 recurrence columns per core
P = 128

JMAX = 4
GROUPS = [1, 1, 2, 4, 4, 4, 4, 4, 4, 2, 1, 1]  # blocks per group, sum 32
assert sum(GROUPS) == N // P


def build_forget_mult(tc, h_d, f_d, z_d, ctx):
    """Emit the per-core Tile program. f_d/z_d/h_d are DRAM APs [N, T] fp16."""
    from concourse import mybir

    nc = tc.nc
    f16 = mybir.dt.float16
    su = mybir.AluOpType.subtract
    mu = mybir.AluOpType.mult

    f_pool = ctx.enter_context(tc.tile_pool(name="fpanel", bufs=4))
    z_pool = ctx.enter_context(tc.tile_pool(name="zpanel", bufs=4))
    b_pool = ctx.enter_context(tc.tile_pool(name="bpanel", bufs=3))
    h_pool = ctx.enter_context(tc.tile_pool(name="hpanel", bufs=3))

    def group_dram(d, r0, gj):
        # rows [P*r0 : P*(r0+gj)] of [N, T] viewed as [p, j, t]
        return d[P * r0 : P * (r0 + gj), :].rearrange("(j p) t -> p j t", p=P)

    r0 = 0
    for g, gj in enumerate(GROUPS):
        fp = f_pool.tile([P, gj, T], f16, tag="fpanel", name=f"fp{g}")
        nc.sync.dma_start(fp[:], group_dram(f_d, r0, gj))
        zp = z_pool.tile([P, gj, T], f16, tag="zpanel", name=f"zp{g}")
        nc.sync.dma_start(zp[:], group_dram(z_d, r0, gj))
        hp = h_pool.tile([P, gj, T], f16, tag="hpanel", name=f"hp{g}")

        # STT + scans chained in pairs of blocks (len-2048 measured fastest
        # at 2.147 ns/elem; 1024 = 2.24, 4096 = 2.55). Column starts
        # self-reset via the host t=0 fixup, so chains are exact.
        bp = b_pool.tile([P, gj, T], f16, tag="bpanel", name=f"bp{g}")
        nc.vector.scalar_tensor_tensor(bp[:], fp[:], 1.0, zp[:], op0=su, op1=mu)
        j0 = 0
        while j0 < gj:
            cw = 2 if gj - j0 >= 2 else 1
            # state = (f * state) - bneg == f*state + (1-f)*z ; fp32 state
            nc.vector.tensor_tensor_scan(
                hp[:, j0 : j0 + cw].rearrange("p j t -> p (j t)"),
                fp[:, j0 : j0 + cw].rearrange("p j t -> p (j t)"),
                bp[:, j0 : j0 + cw].rearrange("p j t -> p (j t)"),
                0.0,
                op0=mu,
                op1=su,
            )
            j0 += cw
        nc.sync.dma_start(group_dram(h_d, r0, gj), hp[:])
        r0 += gj


def build_program():
    import concourse.tile as tile
    from concourse import bacc, mybir

    nc = bacc.Bacc(
        "TRN2",
        target_bir_lowering=False,
        debug=False,
        enable_asserts=False,
        num_devices=NCORES,
    )
    f16 = mybir.dt.float16
    f_d = nc.dram_tensor("f", [N, T], f16, kind="ExternalInput").ap()
    z_d = nc.dram_tensor("z", [N, T], f16, kind="ExternalInput").ap()
    h_d = nc.dram_tensor("h", [N, T], f16, kind="ExternalOutput").ap()
    with tile.TileContext(nc) as tc:
        with ExitStack() as ctx:
            build_forget_mult(tc, h_d, f_d, z_d, ctx)
    nc.compile()
    return nc


_compiled = None


def _get_program():
    global _compiled
    if _compiled is None:
        _compiled = build_program()
    return _compiled


def kernel(f, z, _trace=False):
    from concourse.bass_utils import run_bass_kernel_spmd

    f = np.asarray(f, dtype=np.float32)
    z = np.asarray(z, dtype=np.float32)
    assert f.shape == (T, B, H) and z.shape == (T, B, H)

    nc = _get_program()
    in_maps = []
    for c in range(NCORES):
        # [T, BPC, H] -> [T, N] -> transpose -> [N, T], downcast to fp16
        fc = f[:, c * BPC : (c + 1) * BPC, :].reshape(T, N).T
        zc = z[:, c * BPC : (c + 1) * BPC, :].reshape(T, N).T
        fc16 = np.ascontiguousarray(fc, dtype=np.float16)
        zc16 = np.ascontiguousarray(zc, dtype=np.float16)
        # t=0 fixup: f'_0 = 0, z'_0 = (1-f_0)*z_0: bneg_0 = -(1-f_0)z_0 and
        # h_0 = 0*carry - bneg_0 is exact for any carried state, so scans
        # may chain across column boundaries.
        z0 = (1.0 - fc[:, 0]) * zc[:, 0]  # fp32 math
        fc16[:, 0] = np.float16(0.0)
        zc16[:, 0] = z0.astype(np.float16)
        in_maps.append({"f": fc16, "z": zc16})

    kres = run_bass_kernel_spmd(nc, in_maps, list(range(NCORES)), trace=_trace)
    out = np.empty((T, B, H), dtype=np.float32)
    for c in range(NCORES):
        hc = kres.results[c]["h"]  # [N, T] fp16
        out[:, c * BPC : (c + 1) * BPC, :] = (
            hc.astype(np.float32).reshape(BPC, H, T).transpose(2, 0, 1)
        )
    if _trace:
        return out, kres
    return out
